# revision 9
# baseline (speedup 1.0000x reference)
"""AbsoluteAttention Trainium2 kernel — 8-core SPMD.

Math: the reference's B*T*T*H `scores` tensor is low rank:
    scores[b,t,l,h] = qsum[b,t,h] * (time_q[t,h,:] . time_k[l,h,:])
so
    loading[b,t,h,:] = qsum[b,t,h] * TQ[t,h,:] @ (TK[:,h,:]^T @ kv[b,:,h,:])
which reduces the attention to a per-(batch,head) 64x64 "state" that is the
only cross-row coupling.  The kernel is data-parallel over the 4096 rows of
flattened (B,T): 512 rows per core, with one cross-core exchange (per-batch
groups of 4 cores) of the 128x384 pair-layout state.  Everything else (QKVO
projections, softmax, sigmoid-sum, layernorm) is row-local.

Perf structure (v2):
  - state accumulation matmuls interleave with the KV phase (single PSUM
    bank in pair layout), so the exchange dispatches right after the last
    kv tile instead of after a separate pass.
  - the exchange is an AllGather of the 4 partial states + local sum
    (the cost of an AllReduce is ~1.9x an AllGather of 4x the bytes).
  - K bias is applied with the bias matmul FIRST in the accumulation group
    so exp() never waits on a late bias DMA.
  - the residual add runs on the PE (identity-matmul accumulated into the
    O-projection PSUM group); layernorm reads PSUM directly.
  - Q phase traces after the AllGather dispatch and fills its window.

Matmuls run in fp8/bf16 (fp32 accumulate); validated ~3e-3 max rel error vs
the fp32 reference.
"""
import os
import sys

for _p in ("/opt/trn_rl_repo", "/root/.axon_site/_ro/trn_rl_repo"):
    if os.path.isdir(_p) and _p not in sys.path:
        sys.path.insert(0, _p)

import numpy as np
import ml_dtypes

# Persistent executable cache: lets a fresh process skip the multi-minute
# neuronx-cc compile if this kernel was compiled on this machine before.
try:
    import jax as _jax
    _jax.config.update("jax_compilation_cache_dir",
                       os.path.expanduser("~/.cache/absatt_jax_cache"))
    _jax.config.update("jax_persistent_cache_min_compile_time_secs", 1.0)
    _jax.config.update("jax_persistent_cache_min_entry_size_bytes", 0)
except Exception:
    pass

import concourse.bass as bass
import concourse.bacc as bacc
import concourse.tile as tile
import concourse.mybir as mybir
from concourse.bass_utils import run_bass_kernel_spmd
from concourse.masks import make_identity

BF16 = mybir.dt.bfloat16
F32 = mybir.dt.float32
AF = mybir.ActivationFunctionType
ALU = mybir.AluOpType

B, T, D = 2, 2048, 768
H, DH, DT = 12, 64, 32
J2 = 2 * DT            # 64, time feature dim; H * J2 == D
NCORES = 8
R = (B * T) // NCORES  # 512 rows per core
P = 128
TTILES = R // P        # 4
KTILES = D // P        # 6
NCH = 384              # projection free-dim chunk (2 chunks of 384 = 768)
NGRP = 4               # cores per replica group (one batch)
LN_EPS = 1e-5
REPLICA_GROUPS = [[0, 1, 2, 3], [4, 5, 6, 7]]
F8 = mybir.dt.float8e4
NP_F8 = ml_dtypes.float8_e4m3
# "fp8": QKVO projections in fp8e4 DoubleRow (weights x32, loadT x16, the
# 1/512 unscale folded into the residual+eps since layernorm is
# scale-invariant).  "bf16": everything bf16.
DTYPE_MODE = os.environ.get("ABSATT_DTYPE", "fp8")
WS = 32.0          # fp8 weight scale
LS = 16.0          # fp8 loadT scale
XS = WS * LS       # residual scale in fp8 mode


# --------------------------------------------------------------------------
# device program
# --------------------------------------------------------------------------

def _build_program(flags, debug_dump=False, unroll=1, no_ar=False):
    """flags = (mask_trivial, gb_trivial, bv_zero, bo_zero, qb_uniform, fp8)"""
    mask_trivial, gb_trivial, bv_zero, bo_zero, qb_uniform, fp8 = flags
    MMDT = F8 if fp8 else BF16
    nc = bacc.Bacc("TRN2", target_bir_lowering=False, debug=False,
                   num_devices=NCORES)

    # ---- I/O ----
    xT_d = nc.dram_tensor("xT", [D, R], MMDT, kind="ExternalInput").ap()
    xr_d = nc.dram_tensor("xr", [R, D], BF16, kind="ExternalInput").ap()
    wqT_d = nc.dram_tensor("wqT", [D, D], MMDT, kind="ExternalInput").ap()
    wkT_d = nc.dram_tensor("wkT", [D, D], MMDT, kind="ExternalInput").ap()
    wvT_d = nc.dram_tensor("wvT", [D, D], MMDT, kind="ExternalInput").ap()
    woT_d = nc.dram_tensor("woT", [D, D], MMDT, kind="ExternalInput").ap()
    tk_d = nc.dram_tensor("tk", [R, D], BF16, kind="ExternalInput").ap()
    tqT_d = nc.dram_tensor("tqT", [D, R], BF16, kind="ExternalInput").ap()
    # rows: [-exp(q_bias), bk, bv, bo]
    bias_d = nc.dram_tensor("biases", [4, D], BF16, kind="ExternalInput").ap()
    if qb_uniform:
        # per-partition column: -exp(q_bias[0]) / 2 (tanh bias, see Q phase)
        qb_d = nc.dram_tensor("qb_col", [P, 1], F32, kind="ExternalInput").ap()
    if not mask_trivial:
        # cols: [mask, mask/64]
        mask_d = nc.dram_tensor("maskc", [R, 2], F32, kind="ExternalInput").ap()
    if not gb_trivial:
        gb_d = nc.dram_tensor("gb", [2, D], F32, kind="ExternalInput").ap()
    out_d = nc.dram_tensor("out", [R, D], BF16, kind="ExternalOutput").ap()
    if debug_dump:
        dbg = {
            "d_qsum": nc.dram_tensor("d_qsum", [P, TTILES, H], F32, kind="ExternalOutput").ap(),
            "d_state": nc.dram_tensor("d_state", [P, KTILES * J2], F32, kind="ExternalOutput").ap(),
            "d_tqs": nc.dram_tensor("d_tqs", [P, KTILES, R], BF16, kind="ExternalOutput").ap(),
            "d_loadT": nc.dram_tensor("d_loadT", [P, KTILES, R], BF16, kind="ExternalOutput").ap(),
            "d_kv": nc.dram_tensor("d_kv", [P, TTILES, H, J2], BF16, kind="ExternalOutput").ap(),
            "d_tkr": nc.dram_tensor("d_tkr", [P, TTILES, H, J2], BF16, kind="ExternalOutput").ap(),
            "d_state0": nc.dram_tensor("d_state0", [P, KTILES * J2], BF16, kind="ExternalOutput").ap(),
        }

    with tile.TileContext(nc) as tc:
        with (
            tc.tile_pool(name="per", bufs=1) as per,     # persistent tiles
            tc.tile_pool(name="work", bufs=4) as work,   # per-t-tile working
            tc.tile_pool(name="pj", bufs=5, space="PSUM") as pj_pool,
            tc.tile_pool(name="st", bufs=1, space="PSUM") as st_pool,
            tc.tile_pool(name="auxp", bufs=2, space="PSUM") as aux_pool,
            tc.tile_pool(name="dram", bufs=1, space="DRAM") as dpool,
        ):
            # ---- persistent SBUF ----
            wq = per.tile([P, KTILES, D], MMDT, tag="wq")
            wk = per.tile([P, KTILES, D], MMDT, tag="wk")
            wv = per.tile([P, KTILES, D], MMDT, tag="wv")
            wo = per.tile([P, KTILES, D], MMDT, tag="wo")
            xTs = per.tile([P, KTILES, R], MMDT, tag="xTs")
            xrs = per.tile([P, TTILES, D], BF16, tag="xrs")
            tks = per.tile([P, TTILES, D], BF16, tag="tks")
            tqTs = per.tile([P, KTILES, R], BF16, tag="tqTs")
            biass = per.tile([P, 4, D], BF16, tag="biass")
            ones_l = per.tile([P, P], BF16, tag="ones_l")
            ident = per.tile([P, P], F32, tag="ident")
            identb = per.tile([P, P], BF16, tag="identb")
            e_sb = per.tile([12, KTILES, 2, J2], BF16, tag="e_sb")
            qsum_all = per.tile([P, TTILES, H], F32, tag="qsum_all")
            qsumT = per.tile([12, TTILES, P], BF16, tag="qsumT")
            tqs = per.tile([P, KTILES, R], BF16, tag="tqs")
            loadT = per.tile([P, KTILES, R], MMDT, tag="loadT")
            # pair layout: state[64*g + j, kt*J2 + d] = state[head 2*kt+g][j, d]
            state_sb = per.tile([P, KTILES * J2], BF16, tag="state_sb")
            state_rx = per.tile([P, NGRP, KTILES * J2], BF16, tag="state_rx")
            state_p2 = per.tile([P, 2, KTILES * J2], F32, tag="state_p2")
            state_bf = per.tile([P, KTILES, J2], BF16, tag="state_bf")
            kvs = per.tile([P, TTILES, H, J2], BF16, tag="kvs")
            tkrs = per.tile([P, TTILES, H, J2], BF16, tag="tkrs")
            if not mask_trivial:
                masks = per.tile([P, TTILES, 2], F32, tag="masks")
            if not gb_trivial:
                gbs = per.tile([P, 2, D], F32, tag="gbs")

            # ---- constants built on-chip ----
            nc.vector.memset(ones_l[:], 0.0)
            nc.vector.memset(ones_l[0:1, :], 1.0)
            # only zero the bias rows the matmuls actually read (the ones_l
            # contraction touches every partition; garbage there could be NaN)
            bias_rows = [1]          # bk
            if not qb_uniform:
                bias_rows.append(0)
            if not bv_zero:
                bias_rows.append(2)
            if not bo_zero:
                bias_rows.append(3)
            for r in bias_rows:
                nc.vector.memset(biass[:, r], 0.0)
            make_identity(nc, ident[:])
            nc.scalar.copy(identb[:], ident[:])
            # E[h, kt, g, j] = 1 iff h == 2*kt + g   (broadcast matrix)
            nc.gpsimd.memset(e_sb[:], 0.0)
            nc.gpsimd.affine_select(
                out=e_sb[:], in_=e_sb[:],
                compare_op=ALU.not_equal, fill=1.0, base=0,
                pattern=[[-2, KTILES], [-1, 2], [0, J2]],
                channel_multiplier=1,
            )
            eps_sb = per.tile([P, 1], F32, tag="eps_sb")
            nc.vector.memset(eps_sb[:], LN_EPS * (XS * XS if fp8 else 1.0))
            sq_dummy = per.tile([1, 1], F32, tag="sq_dummy")
            if qb_uniform:
                qb_sb = per.tile([P, 1], F32, tag="qb_sb")

            # ---- input DMAs ----
            # bias rows first (the K-projection accumulation group STARTS with
            # the bias matmul); then x/wk in pair chunks so the first matmuls
            # can begin as soon as the first 256-row chunk lands.
            for r in bias_rows:
                nc.sync.dma_start(biass[0:1, r], bias_d[None, r])
            wkT_r = wkT_d.rearrange("(kt p) n -> p kt n", p=P)
            xT_r = xT_d.rearrange("(kt p) m -> p kt m", p=P)
            for k2 in range(KTILES // 2):  # pair granularity = DoubleRow unit
                ksl = bass.ds(2 * k2, 2)
                nc.sync.dma_start(xTs[:, ksl], xT_r[:, ksl])
                nc.sync.dma_start(wk[:, ksl], wkT_r[:, ksl])
            nc.sync.dma_start(wv[:], wvT_d.rearrange("(kt p) n -> p kt n", p=P))
            nc.sync.dma_start(tks[:], tk_d.rearrange("(tt p) f -> p tt f", p=P))
            nc.sync.dma_start(wq[:], wqT_d.rearrange("(kt p) n -> p kt n", p=P))
            if qb_uniform:
                nc.sync.dma_start(qb_sb[:], qb_d)
            nc.sync.dma_start(tqTs[:], tqT_d.rearrange("(kt p) m -> p kt m", p=P))
            nc.sync.dma_start(wo[:], woT_d.rearrange("(kt p) n -> p kt n", p=P))
            nc.sync.dma_start(xrs[:], xr_d.rearrange("(tt p) f -> p tt f", p=P))
            if not mask_trivial:
                nc.sync.dma_start(masks[:], mask_d.rearrange("(tt p) c -> p tt c", p=P))
            if not gb_trivial:
                gbs_row = per.tile([1, 2, D], F32, tag="gbs_row")
                nc.sync.dma_start(gbs_row[:], gb_d[None, :, :])
                nc.gpsimd.partition_broadcast(gbs[:], gbs_row[:])

            def proj_psum():
                # two single-bank tiles -> finer slot recycling than one
                # [128,2,512] tile (4 x 1-bank slots pipeline deeper)
                return [pj_pool.tile([P, 512], F32, tag="pj", name="pj")
                        for _ in range(2)]

            def run_proj(w_tile, tt, bias_idx, psum_tile, resid=False):
                """psum[:, c, :NCH] = x_tt @ W.T (+ bias row) (+ residual)"""
                for c in range(2):
                    nsl = bass.ds(c * NCH, NCH)
                    pc = psum_tile[c]
                    nmm = KTILES // 2 if fp8 else KTILES
                    last = nmm - 1
                    if bias_idx is not None:
                        # bias FIRST: starts the accumulation group without
                        # gating the activation on a late bias DMA
                        nc.tensor.matmul(
                            pc[:, :NCH], ones_l[:], biass[:, bias_idx, nsl],
                            start=True, stop=False,
                        )
                    lhs = loadT if resid == "o" else xTs
                    if fp8:
                        for k2 in range(KTILES // 2):
                            nc.tensor.matmul(
                                pc[:, :NCH],
                                lhs[:, 2 * k2:2 * k2 + 2, bass.ts(tt, P)],
                                w_tile[:, 2 * k2:2 * k2 + 2, nsl],
                                start=(k2 == 0 and bias_idx is None),
                                stop=(k2 == last and not resid),
                                perf_mode=mybir.MatmulPerfMode.DoubleRow,
                            )
                    else:
                        for kt in range(KTILES):
                            nc.tensor.matmul(
                                pc[:, :NCH],
                                lhs[:, kt, bass.ts(tt, P)],
                                w_tile[:, kt, nsl],
                                start=(kt == 0 and bias_idx is None),
                                stop=(kt == last and not resid),
                            )
                    if resid:
                        # += x_tt (residual) via identity matmul; keeps the
                        # add off the DVE so layernorm reads PSUM directly
                        nc.tensor.matmul(
                            pc[:, :NCH], identb[:], xrs[:, tt, nsl],
                            start=False, stop=True,
                        )

            def _iter_body():
                # ---- state psum: pair layout, accumulated across t-tiles ----
                # state_ps[64*g + j, kt*J2 + d] for head h = 2*kt + g.
                # One PSUM bank; the only start=True is the very first matmul
                # (clears the whole bank's has_written bits), everything else
                # writes fresh regions or accumulates.
                state_ps = st_pool.tile([P, KTILES * J2], F32, tag="state",
                                        name="state_ps")

                # ========== KV phase (per t-tile, state interleaved) ==========
                # The state matmuls for t-tile i are traced AFTER the K/V
                # projections of t-tile i+1: the PE stream is in-order, so
                # putting them inline would serialize K_{i+1} behind the DVE
                # kv chain of tile i.
                def state_mms(tt):
                    # state accumulation straight into the pair layout: even
                    # head -> psum partitions 0-63, odd -> 64-127
                    # (tile_position inferred from out.base_partition)
                    for kt in range(KTILES):
                        for g in range(2):
                            h = 2 * kt + g
                            nc.tensor.matmul(
                                state_ps[bass.ds(g * J2, J2),
                                         bass.ds(kt * J2, J2)],
                                tkrs[:, tt, h],
                                kvs[:, tt, h],
                                start=(tt == 0 and h == 0),
                                stop=(tt == TTILES - 1),
                                tile_position=(0, g * J2),
                                skip_group_check=True,
                            )

                for tt in range(TTILES):
                    # K projection (+bk, bias first)
                    kp = proj_psum()
                    run_proj(wk, tt, 1, kp)
                    # exp(k * mask)  (KTEMP = 1)
                    ek = work.tile([P, D], F32, tag="ek")
                    denom = work.tile([P, H], F32, tag="denom")
                    recip = work.tile([P, H], F32, tag="recip")
                    for c in range(2):  # per-chunk: reduce starts sooner
                        csl = bass.ds(c * 6, 6)
                        nc.scalar.activation(
                            out=ek[:, bass.ds(c * NCH, NCH)],
                            in_=kp[c][:, :NCH],
                            func=AF.Exp,
                            scale=((1.0 / WS if fp8 else 1.0) if mask_trivial
                                   else masks[:, tt, 0:1]),
                        )
                        nc.vector.reduce_sum(
                            denom[:, csl],
                            ek[:, bass.ds(c * NCH, NCH)].rearrange(
                                "p (h j) -> p h j", j=J2),
                            axis=mybir.AxisListType.X,
                        )
                        if fp8:  # kv carries the W-scale; fold 1/WS via denom
                            nc.vector.tensor_scalar(
                                denom[:, csl], denom[:, csl], WS, None, ALU.mult)
                        nc.vector.reciprocal(recip[:, csl], denom[:, csl])
                        # tkr = tk * recip (broadcast over j) -- on gpsimd to
                        # keep DVE free for the kv/reduce chain
                        nc.gpsimd.tensor_tensor(
                            tkrs[:, tt, csl],
                            tks[:, tt].rearrange("p (h j) -> p h j", j=J2)[:, csl],
                            recip[:, csl, None].to_broadcast((P, 6, J2)),
                            ALU.mult,
                        )

                    # V projection (+bv)
                    vp = proj_psum()
                    run_proj(wv, tt, None if bv_zero else 2, vp)
                    if tt > 0:
                        state_mms(tt - 1)
                    # kv_raw = ek * v (per chunk: frees each psum bank asap)
                    kvflat = kvs[:, tt].rearrange("p h j -> p (h j)")
                    for c in range(2):
                        nsl = bass.ds(c * NCH, NCH)
                        nc.vector.tensor_tensor(
                            kvflat[:, nsl], ek[:, nsl], vp[c][:, :NCH],
                            ALU.mult,
                        )
                state_mms(TTILES - 1)

                # ========== state exchange (AllGather + local sum) ==========
                nc.scalar.copy(state_sb[:], state_ps[:])
                if debug_dump:
                    nc.sync.dma_start(dbg["d_state0"], state_sb[:])
                bounce_in = dpool.tile([P, KTILES * J2], BF16)
                bounce_out = dpool.tile([NGRP * P, KTILES * J2], BF16)
                nc.sync.dma_start(bounce_in[:], state_sb[:])
                if not no_ar:  # timing-variant escape hatch
                    nc.gpsimd.collective_compute(
                        "AllGather", ALU.bypass,
                        replica_groups=REPLICA_GROUPS,
                        ins=[bounce_in.opt()],
                        outs=[bounce_out.opt()],
                    )
                # NOTE: the gather readback + sum are traced AFTER the Q
                # phase.  DVE/ACT streams execute in program order, so any
                # AG-dependent op traced here would stall the whole Q phase
                # behind the collective.

                # ========== Q phase (overlaps the AllGather) ==========
                for tt in range(TTILES):
                    qp = proj_psum()
                    run_proj(wq, tt, None if qb_uniform else 0, qp)
                    # sigmoid(x) = 0.5 + 0.5*tanh(x/2);   qsum = sum_d sigmoid
                    # uniform q_bias folds into the tanh bias: tanh((q-e)/2)
                    qt = work.tile([P, D], F32, tag="qt")
                    tsum = work.tile([P, H], F32, tag="tsum")
                    for c in range(2):
                        nc.scalar.activation(
                            out=qt[:, bass.ds(c * NCH, NCH)],
                            in_=qp[c][:, :NCH],
                            func=AF.Tanh, scale=0.5 / (WS if fp8 else 1.0),
                            bias=(qb_sb[:] if qb_uniform else 0.0),
                        )
                        nc.vector.reduce_sum(
                            tsum[:, bass.ds(c * 6, 6)],
                            qt[:, bass.ds(c * NCH, NCH)].rearrange(
                                "p (h j) -> p h j", j=J2),
                            axis=mybir.AxisListType.X,
                        )
                    # qsum/DH (*mask) = (tsum*0.5 + 32) * mask / 64
                    if mask_trivial:
                        nc.vector.tensor_scalar(
                            qsum_all[:, tt], tsum[:], 0.5 / DH, 32.0 / DH,
                            ALU.mult, ALU.add,
                        )
                    else:
                        tmp = work.tile([P, H], F32, tag="qtmp")
                        nc.vector.tensor_scalar(
                            tmp[:], tsum[:], 0.5, 32.0, ALU.mult, ALU.add)
                        nc.vector.tensor_scalar(
                            qsum_all[:, tt], tmp[:], masks[:, tt, 1:2], None,
                            ALU.mult,
                        )

                # qsumT[h, tt*128+t] via PE transpose (4 slices, one copy)
                tr_full = aux_pool.tile([P, 512], F32, tag="aux", name="tr")
                for tt in range(TTILES):
                    nc.tensor.transpose(
                        tr_full[:H, bass.ts(tt, P)], qsum_all[:, tt], ident[:])
                nc.scalar.copy(
                    qsumT[:].rearrange("h tt t -> h (tt t)"), tr_full[:H, :])

                # TQs[(h,j), t] = tqT * qsumT[h, t]  (broadcast via E-matmul)
                for kt in range(KTILES):
                    qe = aux_pool.tile([P, 512], F32, tag="aux", name="qe")
                    nc.tensor.matmul(
                        qe[:, :R],
                        e_sb[:, kt].rearrange("h g j -> h (g j)"),
                        qsumT[:].rearrange("h tt t -> h (tt t)"),
                        start=True, stop=True,
                    )
                    qeb = work.tile([P, R], BF16, tag="qeb")
                    nc.scalar.copy(qeb[:], qe[:, :R])
                    nc.vector.tensor_tensor(
                        tqs[:, kt], tqTs[:, kt], qeb[:], ALU.mult)

                # swap the ACT table to the sqrt set now, under the collective
                # window, so the first layernorm Sqrt doesn't eat the
                # LoadActFuncSet latency (Copy lives in every set)
                nc.scalar.activation(sq_dummy[:], eps_sb[0:1, 0:1], AF.Sqrt)

                # ---- gather readback: 4 partial states -> local sum ----
                src_bounce = (bounce_in if no_ar else bounce_out)
                if no_ar:
                    for r in range(NGRP):
                        nc.sync.dma_start(state_rx[:, r], src_bounce[:])
                else:
                    nc.sync.dma_start(
                        state_rx[:],
                        src_bounce[:].rearrange("(r p) f -> p r f", p=P))

                # ========== loadingT (per kt: sum partials, matmul) ==========
                rx4 = state_rx[:].rearrange("p r (kt d) -> p r kt d", d=J2)
                p24 = state_p2[:].rearrange("p r (kt d) -> p r kt d", d=J2)
                for kt in range(KTILES):
                    nc.vector.tensor_tensor(
                        p24[:, :, kt], rx4[:, 0:2, kt], rx4[:, 2:4, kt],
                        ALU.add)
                    nc.vector.tensor_tensor(
                        state_bf[:, kt], p24[:, 0, kt], p24[:, 1, kt], ALU.add)
                    lt = aux_pool.tile([P, 512], F32, tag="aux", name="lt")
                    nc.tensor.matmul(
                        lt[0:J2, :R], state_bf[0:J2, kt], tqs[0:J2, kt, :],
                        start=True, stop=True, tile_position=(0, 0),
                    )
                    nc.tensor.matmul(
                        lt[J2:P, :R], state_bf[J2:P, kt], tqs[J2:P, kt, :],
                        start=True, stop=True, tile_position=(J2, J2),
                    )
                    # fp8 cast via Copy activation (present in every ACT
                    # table set -- keeps the sqrt set resident)
                    nc.scalar.activation(loadT[:, kt], lt[:, :R], AF.Copy,
                                         scale=LS if fp8 else 1.0)

                if debug_dump:
                    nc.sync.dma_start(dbg["d_qsum"], qsum_all[:])
                    nc.sync.dma_start(dbg["d_state"],
                                      state_bf[:].rearrange("p kt d -> p (kt d)"))
                    nc.sync.dma_start(dbg["d_tqs"], tqs[:])
                    nc.sync.dma_start(dbg["d_loadT"], loadT[:])
                    nc.sync.dma_start(dbg["d_kv"], kvs[:])
                    nc.sync.dma_start(dbg["d_tkr"], tkrs[:])

                # ========== O projection + residual + LN ==========
                for tt in range(TTILES):
                    op = proj_psum()
                    run_proj(wo, tt, None if bo_zero else 3, op, resid="o")
                    # layernorm straight off PSUM (residual already added)
                    stats = work.tile([P, 2, 6], F32, tag="stats")
                    for c in range(2):
                        nc.vector.bn_stats(stats[:, c], op[c][:, :NCH])
                    mv = work.tile([P, 2], F32, tag="mv")
                    nc.vector.bn_aggr(mv[:], stats[:])
                    std = work.tile([P, 1], F32, tag="std")
                    nc.scalar.activation(std[:], mv[:, 1:2], AF.Sqrt, bias=eps_sb[:])
                    outt = work.tile([P, D], BF16, tag="outt")
                    rstd = work.tile([P, 1], F32, tag="rstd")
                    nc.vector.reciprocal(rstd[:], std[:])
                    # nmr = -mean * rstd; normalize runs on ACT as
                    # Copy(psum * rstd + nmr) to keep the DVE free
                    nmr = work.tile([P, 1], F32, tag="nmr")
                    nc.vector.scalar_tensor_tensor(
                        nmr[:], mv[:, 0:1], -1.0, rstd[:], ALU.mult, ALU.mult)
                    for c in range(2):
                        nsl = bass.ds(c * NCH, NCH)
                        nc.scalar.activation(
                            outt[:, nsl], op[c][:, :NCH], AF.Identity,
                            scale=rstd[:], bias=nmr[:],
                        )
                    if not gb_trivial:
                        nc.vector.tensor_tensor(outt[:], outt[:], gbs[:, 0], ALU.mult)
                        nc.vector.tensor_tensor(outt[:], outt[:], gbs[:, 1], ALU.add)
                    nc.sync.dma_start(
                        out_d.rearrange("(tt p) f -> p tt f", p=P)[:, tt], outt[:])


            for _it in range(unroll):
                _iter_body()

    nc.compile()
    return nc


_PROGRAM_CACHE = {}


def _get_program(flags):
    if flags not in _PROGRAM_CACHE:
        _PROGRAM_CACHE[flags] = _build_program(flags)
    return _PROGRAM_CACHE[flags]


# --------------------------------------------------------------------------
# host side
# --------------------------------------------------------------------------

def _time_tensors(time_angle, head_time_delta):
    """time_q/time_k exactly as the reference computes them (f32 angles,
    accurate trig), returned as [T, H*J2] float32."""
    ta = np.asarray(time_angle, np.float32)
    delta = np.asarray(head_time_delta, np.float32)
    pos = np.arange(T, dtype=np.float32)[:, None, None]
    inv = np.float32(1.0 / np.sqrt(np.float32(DH)))

    def gt(d):
        ang = (pos + d) * ta[None]          # [T, H, DT] fp32 (matches ref)
        a64 = ang.astype(np.float64)
        c, s = np.cos(a64), np.sin(a64)
        return (np.concatenate([c + s, c - s], axis=-1) * np.float64(inv)
                ).astype(np.float32)        # [T, H, J2]

    tq = gt(delta[None, :, None]).reshape(T, H * J2)
    tk = gt(np.float32(0.0)).reshape(T, H * J2)
    return tq, tk


def prepare_inputs(states, attention_mask, Wq, Wk, bk, Wv, bv, Wo, bo, q_bias,
                   time_angle, head_time_delta, ln_gamma, ln_beta):
    f32 = np.float32
    bf16 = ml_dtypes.bfloat16
    states = np.asarray(states, f32)
    mask = np.asarray(attention_mask)
    Wq, Wk, Wv, Wo = (np.asarray(w, f32) for w in (Wq, Wk, Wv, Wo))
    bk, bv, bo, q_bias = (np.asarray(v, f32) for v in (bk, bv, bo, q_bias))
    ln_gamma, ln_beta = np.asarray(ln_gamma, f32), np.asarray(ln_beta, f32)

    mask_trivial = bool(np.all(mask == 1))
    gb_trivial = bool(np.all(ln_gamma == 1.0) and np.all(ln_beta == 0.0))
    bv_zero = bool(np.all(bv == 0.0))
    bo_zero = bool(np.all(bo == 0.0))
    qb_uniform = bool(np.all(q_bias == q_bias[0]))
    fp8 = DTYPE_MODE == "fp8"
    flags = (mask_trivial, gb_trivial, bv_zero, bo_zero, qb_uniform, fp8)

    if fp8:
        mmdt = NP_F8
        ws, xs = np.float32(WS), np.float32(XS)
        bscale = np.array([WS, WS, WS, XS], f32)[:, None]
    else:
        mmdt = bf16
        ws, xs = np.float32(1.0), np.float32(1.0)
        bscale = np.ones((4, 1), f32)
    wqT = np.ascontiguousarray(Wq.T * ws).astype(mmdt)
    wkT = np.ascontiguousarray(Wk.T * ws).astype(mmdt)
    wvT = np.ascontiguousarray(Wv.T * ws).astype(mmdt)
    woT = np.ascontiguousarray(Wo.T * ws).astype(mmdt)
    biases = (np.stack([-np.exp(q_bias), bk, bv, bo]) * bscale).astype(bf16)

    tq, tk = _time_tensors(time_angle, head_time_delta)

    xf = states.reshape(B * T, D)
    maskf = mask.reshape(B * T).astype(f32)

    in_maps = []
    for c in range(NCORES):
        rows = slice(c * R, (c + 1) * R)
        tpos = slice((c % 4) * R, (c % 4) * R + R)
        m = {
            "xT": np.ascontiguousarray(xf[rows].T).astype(mmdt),
            "xr": np.ascontiguousarray(xf[rows] * xs).astype(bf16),
            "wqT": wqT, "wkT": wkT, "wvT": wvT, "woT": woT,
            "tk": np.ascontiguousarray(tk[tpos]).astype(bf16),
            "tqT": np.ascontiguousarray(tq[tpos].T).astype(bf16),
            "biases": biases,
        }
        if qb_uniform:
            m["qb_col"] = np.full((P, 1), -np.exp(q_bias[0]) / 2.0, f32)
        if not mask_trivial:
            mc = maskf[rows]
            m["maskc"] = np.stack([mc / (WS if fp8 else 1.0), mc / DH],
                                  axis=1).astype(f32)
        if not gb_trivial:
            m["gb"] = np.stack([ln_gamma, ln_beta]).astype(f32)
        in_maps.append(m)
    return flags, in_maps


def run(inputs, trace=False, trace_kwargs=None):
    flags, in_maps = prepare_inputs(**inputs)
    nc = _get_program(flags)
    res = run_bass_kernel_spmd(
        nc, in_maps, core_ids=list(range(NCORES)),
        trace=trace, **(trace_kwargs or {}))
    full = np.concatenate([np.asarray(res.results[c]["out"])
                           for c in range(NCORES)],
                          axis=0).reshape(B, T, D).astype(np.float32)
    return full, res


def kernel(**inputs):
    out, _ = run(inputs)
    return out


if __name__ == "__main__":
    rng = np.random.default_rng(0)
    fake = {
        "states": rng.standard_normal((B, T, D), dtype=np.float32),
        "attention_mask": np.ones((B, T), np.int32),
        "Wq": rng.standard_normal((D, D), dtype=np.float32) * 0.02,
        "Wk": rng.standard_normal((D, D), dtype=np.float32) * 0.02,
        "bk": rng.standard_normal((D,), dtype=np.float32) * 0.02,
        "Wv": rng.standard_normal((D, D), dtype=np.float32) * 0.02,
        "bv": np.zeros((D,), np.float32),
        "Wo": rng.standard_normal((D, D), dtype=np.float32) * 0.02,
        "bo": np.zeros((D,), np.float32),
        "q_bias": np.zeros((D,), np.float32),
        "time_angle": (rng.random((H, DT), dtype=np.float32) ** 10 + 1e-8),
        "head_time_delta": rng.random((H,), dtype=np.float32),
        "ln_gamma": np.ones((D,), np.float32),
        "ln_beta": np.zeros((D,), np.float32),
    }
    out = kernel(**fake)
    print("kernel ran, out shape", out.shape, "finite:", np.isfinite(out).all())


# revision 15
# speedup vs baseline: 1.2708x; 1.2708x over previous
"""AbsoluteAttention Trainium2 kernel — 8-core SPMD.

Math: the reference's B*T*T*H `scores` tensor is low rank:
    scores[b,t,l,h] = qsum[b,t,h] * (time_q[t,h,:] . time_k[l,h,:])
so
    loading[b,t,h,:] = qsum[b,t,h] * TQ[t,h,:] @ (TK[:,h,:]^T @ kv[b,:,h,:])
which reduces the attention to a per-(batch,head) 64x64 "state" that is the
only cross-row coupling.  The kernel is data-parallel over the 4096 rows of
flattened (B,T): 512 rows per core, with one cross-core exchange (per-batch
groups of 4 cores) of the 128x384 pair-layout state.  Everything else (QKVO
projections, softmax, sigmoid-sum, layernorm) is row-local.

Perf structure (v2):
  - state accumulation matmuls interleave with the KV phase (single PSUM
    bank in pair layout), so the exchange dispatches right after the last
    kv tile instead of after a separate pass.
  - the exchange is an AllGather of the 4 partial states + local sum
    (the cost of an AllReduce is ~1.9x an AllGather of 4x the bytes).
  - K bias is applied with the bias matmul FIRST in the accumulation group
    so exp() never waits on a late bias DMA.
  - the residual add runs on the PE (identity-matmul accumulated into the
    O-projection PSUM group); layernorm reads PSUM directly.
  - Q phase traces after the AllGather dispatch and fills its window.

Matmuls run in fp8/bf16 (fp32 accumulate); validated ~3e-3 max rel error vs
the fp32 reference.
"""
import os
import sys

for _p in ("/opt/trn_rl_repo", "/root/.axon_site/_ro/trn_rl_repo"):
    if os.path.isdir(_p) and _p not in sys.path:
        sys.path.insert(0, _p)

import numpy as np
import ml_dtypes

# Persistent executable cache: lets a fresh process skip the multi-minute
# neuronx-cc compile if this kernel was compiled on this machine before.
try:
    import jax as _jax
    _jax.config.update("jax_compilation_cache_dir",
                       os.path.expanduser("~/.cache/absatt_jax_cache"))
    _jax.config.update("jax_persistent_cache_min_compile_time_secs", 1.0)
    _jax.config.update("jax_persistent_cache_min_entry_size_bytes", 0)
except Exception:
    pass

import concourse.bass as bass
import concourse.bacc as bacc
import concourse.tile as tile
import concourse.mybir as mybir
from concourse.bass_utils import run_bass_kernel_spmd
from concourse.masks import make_identity

BF16 = mybir.dt.bfloat16
F32 = mybir.dt.float32
AF = mybir.ActivationFunctionType
ALU = mybir.AluOpType

B, T, D = 2, 2048, 768
H, DH, DT = 12, 64, 32
J2 = 2 * DT            # 64, time feature dim; H * J2 == D
NCORES = 8
R = (B * T) // NCORES  # 512 rows per core
P = 128
TTILES = R // P        # 4
KTILES = D // P        # 6
NCH = 384              # projection free-dim chunk (2 chunks of 384 = 768)
NGRP = 4               # cores per replica group (one batch)
LN_EPS = 1e-5
REPLICA_GROUPS = [[0, 1, 2, 3], [4, 5, 6, 7]]
F8 = mybir.dt.float8e4
NP_F8 = ml_dtypes.float8_e4m3
# "fp8": QKVO projections in fp8e4 DoubleRow (weights x32, loadT x16, the
# 1/512 unscale folded into the residual+eps since layernorm is
# scale-invariant).  "bf16": everything bf16.
DTYPE_MODE = os.environ.get("ABSATT_DTYPE", "fp8")
WS = 32.0          # fp8 weight scale
LS = 16.0          # fp8 loadT scale
XS = WS * LS       # residual scale in fp8 mode


# --------------------------------------------------------------------------
# device program
# --------------------------------------------------------------------------

def _build_program(flags, debug_dump=False, unroll=1, no_ar=False):
    """flags = (mask_trivial, gb_trivial, bv_zero, bo_zero, qb_uniform, fp8)"""
    mask_trivial, gb_trivial, bv_zero, bo_zero, qb_uniform, fp8 = flags
    MMDT = F8 if fp8 else BF16
    nc = bacc.Bacc("TRN2", target_bir_lowering=False, debug=False,
                   num_devices=NCORES)

    # ---- I/O ----
    xT_d = nc.dram_tensor("xT", [D, R], MMDT, kind="ExternalInput").ap()
    xr_d = nc.dram_tensor("xr", [R, D], BF16, kind="ExternalInput").ap()
    wqT_d = nc.dram_tensor("wqT", [D, D], MMDT, kind="ExternalInput").ap()
    wkT_d = nc.dram_tensor("wkT", [D, D], MMDT, kind="ExternalInput").ap()
    wvT_d = nc.dram_tensor("wvT", [D, D], MMDT, kind="ExternalInput").ap()
    woT_d = nc.dram_tensor("woT", [D, D], MMDT, kind="ExternalInput").ap()
    tk_d = nc.dram_tensor("tk", [R, D], BF16, kind="ExternalInput").ap()
    tqT_d = nc.dram_tensor("tqT", [D, R], BF16, kind="ExternalInput").ap()
    # rows: [-exp(q_bias), bk, bv, bo]
    bias_d = nc.dram_tensor("biases", [4, D], BF16, kind="ExternalInput").ap()
    if qb_uniform:
        # per-partition column: -exp(q_bias[0]) / 2 (tanh bias, see Q phase)
        qb_d = nc.dram_tensor("qb_col", [P, 1], F32, kind="ExternalInput").ap()
    if not mask_trivial:
        # cols: [mask, mask/64]
        mask_d = nc.dram_tensor("maskc", [R, 2], F32, kind="ExternalInput").ap()
    if not gb_trivial:
        gb_d = nc.dram_tensor("gb", [2, D], F32, kind="ExternalInput").ap()
    out_d = nc.dram_tensor("out", [R, D], BF16, kind="ExternalOutput").ap()
    if debug_dump:
        dbg = {
            "d_qsum": nc.dram_tensor("d_qsum", [P, TTILES, H], F32, kind="ExternalOutput").ap(),
            "d_state": nc.dram_tensor("d_state", [P, KTILES * J2], F32, kind="ExternalOutput").ap(),
            "d_tqs": nc.dram_tensor("d_tqs", [P, KTILES, R], BF16, kind="ExternalOutput").ap(),
            "d_loadT": nc.dram_tensor("d_loadT", [P, KTILES, R], BF16, kind="ExternalOutput").ap(),
            "d_kv": nc.dram_tensor("d_kv", [P, TTILES, H, J2], BF16, kind="ExternalOutput").ap(),
            "d_tkr": nc.dram_tensor("d_tkr", [P, TTILES, H, J2], BF16, kind="ExternalOutput").ap(),
            "d_state0": nc.dram_tensor("d_state0", [P, KTILES * J2], BF16, kind="ExternalOutput").ap(),
        }

    with tile.TileContext(nc) as tc:
        with (
            tc.tile_pool(name="per", bufs=1) as per,     # persistent tiles
            tc.tile_pool(name="work", bufs=4) as work,   # per-t-tile working
            tc.tile_pool(name="pj", bufs=7, space="PSUM") as pj_pool,
            tc.tile_pool(name="st", bufs=1, space="PSUM") as st_pool,
            tc.tile_pool(name="dram", bufs=1, space="DRAM") as dpool,
        ):
            # ---- persistent SBUF ----
            wq = per.tile([P, KTILES, D], MMDT, tag="wq")
            wk = per.tile([P, KTILES, D], MMDT, tag="wk")
            wv = per.tile([P, KTILES, D], MMDT, tag="wv")
            wo = per.tile([P, KTILES, D], MMDT, tag="wo")
            xTs = per.tile([P, KTILES, R], MMDT, tag="xTs")
            xrs = per.tile([P, TTILES, D], BF16, tag="xrs")
            tks = per.tile([P, TTILES, D], BF16, tag="tks")
            tqTs = per.tile([P, KTILES, R], BF16, tag="tqTs")
            biass = per.tile([P, 4, D], BF16, tag="biass")
            ones_l = per.tile([P, P], BF16, tag="ones_l")
            ident = per.tile([P, P], F32, tag="ident")
            identb = per.tile([P, P], BF16, tag="identb")
            e_sb = per.tile([12, KTILES, 2, J2], BF16, tag="e_sb")
            qsum_all = per.tile([P, TTILES, H], F32, tag="qsum_all")
            qsumT = per.tile([12, TTILES, P], BF16, tag="qsumT")
            tqs = per.tile([P, KTILES, R], BF16, tag="tqs")
            loadT = per.tile([P, KTILES, R], MMDT, tag="loadT")
            # pair layout: state[64*g + j, kt*J2 + d] = state[head 2*kt+g][j, d]
            state_sb = per.tile([P, KTILES * J2], BF16, tag="state_sb")
            state_rx = per.tile([P, NGRP, KTILES * J2], BF16, tag="state_rx")
            state_p2 = per.tile([P, 2, KTILES * J2], F32, tag="state_p2")
            state_bf = per.tile([P, KTILES, J2], BF16, tag="state_bf")
            kvs = per.tile([P, TTILES, H, J2], BF16, tag="kvs")
            tkrs = per.tile([P, TTILES, H, J2], BF16, tag="tkrs")
            if not mask_trivial:
                masks = per.tile([P, TTILES, 2], F32, tag="masks")
            if not gb_trivial:
                gbs = per.tile([P, 2, D], F32, tag="gbs")

            # ---- constants built on-chip ----
            nc.vector.memset(ones_l[:], 0.0)
            nc.vector.memset(ones_l[0:1, :], 1.0)
            # only zero the bias rows the matmuls actually read (the ones_l
            # contraction touches every partition; garbage there could be NaN)
            bias_rows = [1]          # bk
            if not qb_uniform:
                bias_rows.append(0)
            if not bv_zero:
                bias_rows.append(2)
            if not bo_zero:
                bias_rows.append(3)
            for r in bias_rows:
                nc.vector.memset(biass[:, r], 0.0)
            make_identity(nc, ident[:])
            nc.scalar.copy(identb[:], ident[:])
            # E[h, kt, g, j] = 1 iff h == 2*kt + g   (broadcast matrix)
            nc.gpsimd.memset(e_sb[:], 0.0)
            nc.gpsimd.affine_select(
                out=e_sb[:], in_=e_sb[:],
                compare_op=ALU.not_equal, fill=1.0, base=0,
                pattern=[[-2, KTILES], [-1, 2], [0, J2]],
                channel_multiplier=1,
            )
            eps_sb = per.tile([P, 1], F32, tag="eps_sb")
            nc.vector.memset(eps_sb[:], LN_EPS * (XS * XS if fp8 else 1.0))
            sq_dummy = per.tile([1, 1], F32, tag="sq_dummy")
            if qb_uniform:
                qb_sb = per.tile([P, 1], F32, tag="qb_sb")
                qb2 = per.tile([P, 1], F32, tag="qb2")

            # ---- input DMAs ----
            # bias rows first (the K-projection accumulation group STARTS with
            # the bias matmul); then x/wk in pair chunks so the first matmuls
            # can begin as soon as the first 256-row chunk lands.
            for r in bias_rows:
                nc.sync.dma_start(biass[0:1, r], bias_d[None, r])
            # single descriptors: the first exp needs the FULL contraction
            # (all k-tiles of x and wk), so chunked DMAs only add ring
            # descriptor latency
            nc.sync.dma_start(xTs[:], xT_d.rearrange("(kt p) m -> p kt m", p=P))
            nc.sync.dma_start(wk[:], wkT_d.rearrange("(kt p) n -> p kt n", p=P))
            nc.sync.dma_start(wv[:], wvT_d.rearrange("(kt p) n -> p kt n", p=P))
            nc.sync.dma_start(tks[:], tk_d.rearrange("(tt p) f -> p tt f", p=P))
            nc.sync.dma_start(wq[:], wqT_d.rearrange("(kt p) n -> p kt n", p=P))
            if qb_uniform:
                nc.sync.dma_start(qb_sb[:], qb_d)
            nc.sync.dma_start(tqTs[:], tqT_d.rearrange("(kt p) m -> p kt m", p=P))
            nc.sync.dma_start(wo[:], woT_d.rearrange("(kt p) n -> p kt n", p=P))
            nc.sync.dma_start(xrs[:], xr_d.rearrange("(tt p) f -> p tt f", p=P))
            if not mask_trivial:
                nc.sync.dma_start(masks[:], mask_d.rearrange("(tt p) c -> p tt c", p=P))
            if not gb_trivial:
                gbs_row = per.tile([1, 2, D], F32, tag="gbs_row")
                nc.sync.dma_start(gbs_row[:], gb_d[None, :, :])
                nc.gpsimd.partition_broadcast(gbs[:], gbs_row[:])

            def proj_psum():
                # two single-bank tiles -> finer slot recycling than one
                # [128,2,512] tile (4 x 1-bank slots pipeline deeper)
                return [pj_pool.tile([P, 512], F32, tag="pj", name="pj")
                        for _ in range(2)]

            def run_proj(w_tile, tt, bias_idx, psum_tile, resid=False):
                """psum[:, c, :NCH] = x_tt @ W.T (+ bias row) (+ residual)"""
                for c in range(2):
                    nsl = bass.ds(c * NCH, NCH)
                    pc = psum_tile[c]
                    nmm = KTILES // 2 if fp8 else KTILES
                    last = nmm - 1
                    if bias_idx is not None:
                        # bias FIRST: starts the accumulation group without
                        # gating the activation on a late bias DMA
                        nc.tensor.matmul(
                            pc[:, :NCH], ones_l[:], biass[:, bias_idx, nsl],
                            start=True, stop=False,
                        )
                    lhs = loadT if resid == "o" else xTs
                    if fp8:
                        for k2 in range(KTILES // 2):
                            nc.tensor.matmul(
                                pc[:, :NCH],
                                lhs[:, 2 * k2:2 * k2 + 2, bass.ts(tt, P)],
                                w_tile[:, 2 * k2:2 * k2 + 2, nsl],
                                start=(k2 == 0 and bias_idx is None),
                                stop=(k2 == last and not resid),
                                perf_mode=mybir.MatmulPerfMode.DoubleRow,
                            )
                    else:
                        for kt in range(KTILES):
                            nc.tensor.matmul(
                                pc[:, :NCH],
                                lhs[:, kt, bass.ts(tt, P)],
                                w_tile[:, kt, nsl],
                                start=(kt == 0 and bias_idx is None),
                                stop=(kt == last and not resid),
                            )
                    if resid:
                        # += x_tt (residual) via identity matmul; keeps the
                        # add off the DVE so layernorm reads PSUM directly
                        nc.tensor.matmul(
                            pc[:, :NCH], identb[:], xrs[:, tt, nsl],
                            start=False, stop=True,
                        )

            def _iter_body():
                # ---- state psum: pair layout, accumulated across t-tiles ----
                # state_ps[64*g + j, kt*J2 + d] for head h = 2*kt + g.
                # One PSUM bank; the only start=True is the very first matmul
                # (clears the whole bank's has_written bits), everything else
                # writes fresh regions or accumulates.
                state_ps = st_pool.tile([P, KTILES * J2], F32, tag="state",
                                        name="state_ps")

                # ========== KV phase (per t-tile, state interleaved) ==========
                # The state matmuls for t-tile i are traced AFTER the K/V
                # projections of t-tile i+1: the PE stream is in-order, so
                # putting them inline would serialize K_{i+1} behind the DVE
                # kv chain of tile i.
                def state_mms(tt):
                    # state accumulation straight into the pair layout: even
                    # head -> psum partitions 0-63, odd -> 64-127
                    # (tile_position inferred from out.base_partition)
                    for kt in range(KTILES):
                        for g in range(2):
                            h = 2 * kt + g
                            nc.tensor.matmul(
                                state_ps[bass.ds(g * J2, J2),
                                         bass.ds(kt * J2, J2)],
                                tkrs[:, tt, h],
                                kvs[:, tt, h],
                                start=(tt == 0 and h == 0),
                                stop=(tt == TTILES - 1),
                                tile_position=(0, g * J2),
                                skip_group_check=True,
                            )

                for tt in range(TTILES):
                    # K projection (+bk, bias first)
                    kp = proj_psum()
                    run_proj(wk, tt, 1, kp)
                    # exp(k * mask)  (KTEMP = 1)
                    ek = work.tile([P, D], F32, tag="ek")
                    denom = work.tile([P, H], F32, tag="denom")
                    recip = work.tile([P, H], F32, tag="recip")
                    for c in range(2):  # per-chunk: reduce starts sooner
                        csl = bass.ds(c * 6, 6)
                        nc.scalar.activation(
                            out=ek[:, bass.ds(c * NCH, NCH)],
                            in_=kp[c][:, :NCH],
                            func=AF.Exp,
                            scale=((1.0 / WS if fp8 else 1.0) if mask_trivial
                                   else masks[:, tt, 0:1]),
                        )
                        nc.vector.reduce_sum(
                            denom[:, csl],
                            ek[:, bass.ds(c * NCH, NCH)].rearrange(
                                "p (h j) -> p h j", j=J2),
                            axis=mybir.AxisListType.X,
                        )
                        if fp8:  # kv carries the W-scale; fold 1/WS via denom
                            nc.vector.tensor_scalar(
                                denom[:, csl], denom[:, csl], WS, None, ALU.mult)
                        nc.vector.reciprocal(recip[:, csl], denom[:, csl])
                        # tkr = tk * recip (broadcast over j) -- on gpsimd
                        # to keep the DVE free for the kv/reduce chain
                        nc.gpsimd.tensor_tensor(
                            tkrs[:, tt, csl],
                            tks[:, tt].rearrange("p (h j) -> p h j", j=J2)[:, csl],
                            recip[:, csl, None].to_broadcast((P, 6, J2)),
                            ALU.mult,
                        )

                    # V projection (+bv)
                    vp = proj_psum()
                    run_proj(wv, tt, None if bv_zero else 2, vp)
                    if tt > 0:
                        state_mms(tt - 1)
                    # kv_raw = ek * v (per chunk: frees each psum bank asap)
                    kvflat = kvs[:, tt].rearrange("p h j -> p (h j)")
                    for c in range(2):
                        nsl = bass.ds(c * NCH, NCH)
                        nc.vector.tensor_tensor(
                            kvflat[:, nsl], ek[:, nsl], vp[c][:, :NCH],
                            ALU.mult,
                        )
                state_mms(TTILES - 1)

                # ========== state exchange (AllGather + local sum) ==========
                nc.scalar.copy(state_sb[:], state_ps[:])
                if qb_uniform:
                    # re-copy the tanh bias AFTER the state copy: gates the
                    # whole Q-phase ACT/DVE chain behind the state critical
                    # path so it can't steal engine time from the KV tail
                    nc.scalar.copy(qb2[:], qb_sb[:])
                if debug_dump:
                    nc.sync.dma_start(dbg["d_state0"], state_sb[:])
                bounce_in = dpool.tile([P, KTILES * J2], BF16)
                bounce_out = dpool.tile([NGRP * P, KTILES * J2], BF16)
                nc.sync.dma_start(bounce_in[:], state_sb[:])
                if not no_ar:  # timing-variant escape hatch
                    nc.gpsimd.collective_compute(
                        "AllGather", ALU.bypass,
                        replica_groups=REPLICA_GROUPS,
                        ins=[bounce_in.opt()],
                        outs=[bounce_out.opt()],
                    )
                # NOTE: the gather readback + sum are traced AFTER the Q
                # phase.  DVE/ACT streams execute in program order, so any
                # AG-dependent op traced here would stall the whole Q phase
                # behind the collective.

                # ========== Q phase (overlaps the AllGather) ==========
                for tt in range(TTILES):
                    qp = proj_psum()
                    run_proj(wq, tt, None if qb_uniform else 0, qp)
                    # sigmoid(x) = 0.5 + 0.5*tanh(x/2);   qsum = sum_d sigmoid
                    # uniform q_bias folds into the tanh bias: tanh((q-e)/2)
                    qt = work.tile([P, D], F32, tag="qt")
                    tsum = work.tile([P, H], F32, tag="tsum")
                    for c in range(2):
                        nc.scalar.activation(
                            out=qt[:, bass.ds(c * NCH, NCH)],
                            in_=qp[c][:, :NCH],
                            func=AF.Tanh, scale=0.5 / (WS if fp8 else 1.0),
                            bias=(qb2[:] if qb_uniform else 0.0),
                        )
                        nc.vector.reduce_sum(
                            tsum[:, bass.ds(c * 6, 6)],
                            qt[:, bass.ds(c * NCH, NCH)].rearrange(
                                "p (h j) -> p h j", j=J2),
                            axis=mybir.AxisListType.X,
                        )
                    # qsum/DH (*mask) = (tsum*0.5 + 32) * mask / 64
                    if mask_trivial:
                        nc.vector.tensor_scalar(
                            qsum_all[:, tt], tsum[:], 0.5 / DH, 32.0 / DH,
                            ALU.mult, ALU.add,
                        )
                    else:
                        tmp = work.tile([P, H], F32, tag="qtmp")
                        nc.vector.tensor_scalar(
                            tmp[:], tsum[:], 0.5, 32.0, ALU.mult, ALU.add)
                        nc.vector.tensor_scalar(
                            qsum_all[:, tt], tmp[:], masks[:, tt, 1:2], None,
                            ALU.mult,
                        )

                # qsumT[h, tt*128+t] via PE transpose (4 slices, one copy)
                tr_full = pj_pool.tile([P, 512], F32, tag="pj", name="tr")
                for tt in range(TTILES):
                    nc.tensor.transpose(
                        tr_full[:H, bass.ts(tt, P)], qsum_all[:, tt], ident[:])
                nc.scalar.copy(
                    qsumT[:].rearrange("h tt t -> h (tt t)"), tr_full[:H, :])

                # TQs[(h,j), t] = tqT * qsumT[h, t]  (broadcast via E-matmul)
                for kt in range(KTILES):
                    qe = pj_pool.tile([P, 512], F32, tag="pj", name="qe")
                    nc.tensor.matmul(
                        qe[:, :R],
                        e_sb[:, kt].rearrange("h g j -> h (g j)"),
                        qsumT[:].rearrange("h tt t -> h (tt t)"),
                        start=True, stop=True,
                    )
                    qeb = work.tile([P, R], BF16, tag="qeb")
                    nc.scalar.copy(qeb[:], qe[:, :R])
                    nc.vector.tensor_tensor(
                        tqs[:, kt], tqTs[:, kt], qeb[:], ALU.mult)
                    if kt == 0:
                        # swap the ACT table to the sqrt set now, under the
                        # collective window (gated on the Q phase via qt so it
                        # isn't hoisted before the exp/tanh phase); the first
                        # layernorm Sqrt then skips the LoadActFuncSet stall
                        nc.scalar.activation(sq_dummy[:], qt[0:1, 0:1], AF.Sqrt)

                # ---- gather readback: 4 partial states -> local sum ----
                src_bounce = (bounce_in if no_ar else bounce_out)
                if no_ar:
                    for r in range(NGRP):
                        nc.sync.dma_start(state_rx[:, r], src_bounce[:])
                else:
                    nc.sync.dma_start(
                        state_rx[:],
                        src_bounce[:].rearrange("(r p) f -> p r f", p=P))

                # ========== loadingT (per kt: sum partials, matmul) ==========
                rx4 = state_rx[:].rearrange("p r (kt d) -> p r kt d", d=J2)
                p24 = state_p2[:].rearrange("p r (kt d) -> p r kt d", d=J2)
                for kt in range(KTILES):
                    nc.vector.tensor_tensor(
                        p24[:, :, kt], rx4[:, 0:2, kt], rx4[:, 2:4, kt],
                        ALU.add)
                    nc.vector.tensor_tensor(
                        state_bf[:, kt], p24[:, 0, kt], p24[:, 1, kt], ALU.add)
                    lt = pj_pool.tile([P, 512], F32, tag="pj", name="lt")
                    nc.tensor.matmul(
                        lt[0:J2, :R], state_bf[0:J2, kt], tqs[0:J2, kt, :],
                        start=True, stop=True, tile_position=(0, 0),
                    )
                    nc.tensor.matmul(
                        lt[J2:P, :R], state_bf[J2:P, kt], tqs[J2:P, kt, :],
                        start=True, stop=True, tile_position=(J2, J2),
                    )
                    # fp8 cast via Copy activation (present in every ACT
                    # table set -- keeps the sqrt set resident)
                    nc.scalar.activation(loadT[:, kt], lt[:, :R], AF.Copy,
                                         scale=LS if fp8 else 1.0)

                if debug_dump:
                    nc.sync.dma_start(dbg["d_qsum"], qsum_all[:])
                    nc.sync.dma_start(dbg["d_state"],
                                      state_bf[:].rearrange("p kt d -> p (kt d)"))
                    nc.sync.dma_start(dbg["d_tqs"], tqs[:])
                    nc.sync.dma_start(dbg["d_loadT"], loadT[:])
                    nc.sync.dma_start(dbg["d_kv"], kvs[:])
                    nc.sync.dma_start(dbg["d_tkr"], tkrs[:])

                # ========== O projection + residual + LN ==========
                for tt in range(TTILES):
                    op = proj_psum()
                    run_proj(wo, tt, None if bo_zero else 3, op, resid="o")
                    # layernorm straight off PSUM (residual already added)
                    stats = work.tile([P, 2, 6], F32, tag="stats")
                    for c in range(2):
                        nc.vector.bn_stats(stats[:, c], op[c][:, :NCH])
                    mv = work.tile([P, 2], F32, tag="mv")
                    nc.vector.bn_aggr(mv[:], stats[:])
                    std = work.tile([P, 1], F32, tag="std")
                    nc.scalar.activation(std[:], mv[:, 1:2], AF.Sqrt, bias=eps_sb[:])
                    outt = work.tile([P, D], BF16, tag="outt")
                    rstd = work.tile([P, 1], F32, tag="rstd")
                    nc.vector.reciprocal(rstd[:], std[:])
                    # nmr = -mean * rstd; normalize runs on ACT as
                    # Copy(psum * rstd + nmr) to keep the DVE free
                    nmr = work.tile([P, 1], F32, tag="nmr")
                    nc.vector.scalar_tensor_tensor(
                        nmr[:], mv[:, 0:1], -1.0, rstd[:], ALU.mult, ALU.mult)
                    nc.vector.tensor_scalar(
                        outt[:, 0:NCH], op[0][:, :NCH], mv[:, 0:1], rstd[:],
                        ALU.subtract, ALU.mult,
                    )
                    nc.scalar.activation(
                        outt[:, NCH:D], op[1][:, :NCH], AF.Identity,
                        scale=rstd[:], bias=nmr[:],
                    )
                    if not gb_trivial:
                        nc.vector.tensor_tensor(outt[:], outt[:], gbs[:, 0], ALU.mult)
                        nc.vector.tensor_tensor(outt[:], outt[:], gbs[:, 1], ALU.add)
                    nc.sync.dma_start(
                        out_d.rearrange("(tt p) f -> p tt f", p=P)[:, tt], outt[:])


            for _it in range(unroll):
                _iter_body()

    nc.compile()
    return nc


_PROGRAM_CACHE = {}


def _get_program(flags):
    if flags not in _PROGRAM_CACHE:
        _PROGRAM_CACHE[flags] = _build_program(flags)
    return _PROGRAM_CACHE[flags]


# --------------------------------------------------------------------------
# host side
# --------------------------------------------------------------------------

def _time_tensors(time_angle, head_time_delta):
    """time_q/time_k exactly as the reference computes them (f32 angles,
    accurate trig), returned as [T, H*J2] float32."""
    ta = np.asarray(time_angle, np.float32)
    delta = np.asarray(head_time_delta, np.float32)
    pos = np.arange(T, dtype=np.float32)[:, None, None]
    inv = np.float32(1.0 / np.sqrt(np.float32(DH)))

    def gt(d):
        ang = (pos + d) * ta[None]          # [T, H, DT] fp32 (matches ref)
        a64 = ang.astype(np.float64)
        c, s = np.cos(a64), np.sin(a64)
        return (np.concatenate([c + s, c - s], axis=-1) * np.float64(inv)
                ).astype(np.float32)        # [T, H, J2]

    tq = gt(delta[None, :, None]).reshape(T, H * J2)
    tk = gt(np.float32(0.0)).reshape(T, H * J2)
    return tq, tk


def prepare_inputs(states, attention_mask, Wq, Wk, bk, Wv, bv, Wo, bo, q_bias,
                   time_angle, head_time_delta, ln_gamma, ln_beta):
    f32 = np.float32
    bf16 = ml_dtypes.bfloat16
    states = np.asarray(states, f32)
    mask = np.asarray(attention_mask)
    Wq, Wk, Wv, Wo = (np.asarray(w, f32) for w in (Wq, Wk, Wv, Wo))
    bk, bv, bo, q_bias = (np.asarray(v, f32) for v in (bk, bv, bo, q_bias))
    ln_gamma, ln_beta = np.asarray(ln_gamma, f32), np.asarray(ln_beta, f32)

    mask_trivial = bool(np.all(mask == 1))
    gb_trivial = bool(np.all(ln_gamma == 1.0) and np.all(ln_beta == 0.0))
    bv_zero = bool(np.all(bv == 0.0))
    bo_zero = bool(np.all(bo == 0.0))
    qb_uniform = bool(np.all(q_bias == q_bias[0]))
    fp8 = DTYPE_MODE == "fp8"
    flags = (mask_trivial, gb_trivial, bv_zero, bo_zero, qb_uniform, fp8)

    if fp8:
        mmdt = NP_F8
        ws, xs = np.float32(WS), np.float32(XS)
        bscale = np.array([WS, WS, WS, XS], f32)[:, None]
    else:
        mmdt = bf16
        ws, xs = np.float32(1.0), np.float32(1.0)
        bscale = np.ones((4, 1), f32)
    wqT = np.ascontiguousarray(Wq.T * ws).astype(mmdt)
    wkT = np.ascontiguousarray(Wk.T * ws).astype(mmdt)
    wvT = np.ascontiguousarray(Wv.T * ws).astype(mmdt)
    woT = np.ascontiguousarray(Wo.T * ws).astype(mmdt)
    biases = (np.stack([-np.exp(q_bias), bk, bv, bo]) * bscale).astype(bf16)

    tq, tk = _time_tensors(time_angle, head_time_delta)

    xf = states.reshape(B * T, D)
    maskf = mask.reshape(B * T).astype(f32)

    in_maps = []
    for c in range(NCORES):
        rows = slice(c * R, (c + 1) * R)
        tpos = slice((c % 4) * R, (c % 4) * R + R)
        m = {
            "xT": np.ascontiguousarray(xf[rows].T).astype(mmdt),
            "xr": np.ascontiguousarray(xf[rows] * xs).astype(bf16),
            "wqT": wqT, "wkT": wkT, "wvT": wvT, "woT": woT,
            "tk": np.ascontiguousarray(tk[tpos]).astype(bf16),
            "tqT": np.ascontiguousarray(tq[tpos].T).astype(bf16),
            "biases": biases,
        }
        if qb_uniform:
            m["qb_col"] = np.full((P, 1), -np.exp(q_bias[0]) / 2.0, f32)
        if not mask_trivial:
            mc = maskf[rows]
            m["maskc"] = np.stack([mc / (WS if fp8 else 1.0), mc / DH],
                                  axis=1).astype(f32)
        if not gb_trivial:
            m["gb"] = np.stack([ln_gamma, ln_beta]).astype(f32)
        in_maps.append(m)
    return flags, in_maps


def run(inputs, trace=False, trace_kwargs=None):
    flags, in_maps = prepare_inputs(**inputs)
    nc = _get_program(flags)
    res = run_bass_kernel_spmd(
        nc, in_maps, core_ids=list(range(NCORES)),
        trace=trace, **(trace_kwargs or {}))
    full = np.concatenate([np.asarray(res.results[c]["out"])
                           for c in range(NCORES)],
                          axis=0).reshape(B, T, D).astype(np.float32)
    return full, res


def kernel(**inputs):
    out, _ = run(inputs)
    return out


if __name__ == "__main__":
    rng = np.random.default_rng(0)
    fake = {
        "states": rng.standard_normal((B, T, D), dtype=np.float32),
        "attention_mask": np.ones((B, T), np.int32),
        "Wq": rng.standard_normal((D, D), dtype=np.float32) * 0.02,
        "Wk": rng.standard_normal((D, D), dtype=np.float32) * 0.02,
        "bk": rng.standard_normal((D,), dtype=np.float32) * 0.02,
        "Wv": rng.standard_normal((D, D), dtype=np.float32) * 0.02,
        "bv": np.zeros((D,), np.float32),
        "Wo": rng.standard_normal((D, D), dtype=np.float32) * 0.02,
        "bo": np.zeros((D,), np.float32),
        "q_bias": np.zeros((D,), np.float32),
        "time_angle": (rng.random((H, DT), dtype=np.float32) ** 10 + 1e-8),
        "head_time_delta": rng.random((H,), dtype=np.float32),
        "ln_gamma": np.ones((D,), np.float32),
        "ln_beta": np.zeros((D,), np.float32),
    }
    out = kernel(**fake)
    print("kernel ran, out shape", out.shape, "finite:", np.isfinite(out).all())
